# revision 33
# baseline (speedup 1.0000x reference)
"""GCN layer (CustomGraphConv) on 8 trn2 NeuronCores via Bass/Tile.

Math: out = D^{-1/2} (A + I) D^{-1/2} @ X @ W + bias
  - A: [N, N] 0/1 symmetric adjacency (f32 input), N = 8192
  - X: [N, 256] f32, W: [256, 256] f32, bias: [256] f32

Sharding: 1D node partition. Core c owns nodes R_c = [c*C, (c+1)*C), C = N/8.
Each core receives the column strip A_aug[:, R_c] (A with self-loops added on
the host, cast to fp8e4 — exact for 0/1 values), the full X^T and W in fp16
(replicated), and the bias broadcast to a [128, 256] f32 tile.

Device program (identical SPMD program on all 8 cores):
  1. Load A strip into SBUF as 16 packed [128, 4C] fp8 tiles (resident:
     A is read from HBM exactly once; 4 k-tiles per DMA amortize the
     per-DMA issue cost).
  2. deg = colsum(strip) on PE: ones[128,128] stationary x A-moving, row 0 of
     psum = degrees of own nodes (complete, by symmetry of A). One
     accumulation group per PSUM bank (hardware restriction).
  3. AllGather degree shards -> deg_full [N]; d = 1/sqrt(deg) (recip+sqrt).
  4. Z = X @ W via PE (X^T chunks as stationary, W as moving), fp16.
  5. Zd = d * Z (per-partition scale, in place).
  6. out_psum = sum_k A[k][:, jt]^T @ Zd[k] (A fp8 stationary, Zd fp16
     moving), jt-outer / k-inner so each PSUM bank hosts one accumulation
     group at a time.
  7. out = d_own * out_psum + bias; per-jt stores.

Toolchain constraints discovered on this stack:
  - walrus rejects >1 semaphore wait per instruction -> _split_dma_waits
    hoists extras onto standalone EventSemaphore instructions.
  - SBUF access patterns must keep the partition dim explicit: t[0, :]
    (rank-dropped) misdrives the DMA; use t[0:1, :].
  - A matmul accumulation group must own its PSUM bank exclusively until
    `stop` (start=True clears the whole bank).
  - fp8 is exact for 0/1 adjacency values and halves DMA + SBUF vs fp16;
    mixed fp8 (stationary) x fp16 (moving) matmuls work.
"""

import numpy as np
import ml_dtypes

import concourse.bass as bass
import concourse.mybir as mybir
import concourse.tile as tile
from concourse.bass_utils import run_bass_kernel_spmd

NCORES = 8
F = 256

f32 = mybir.dt.float32
fp16 = mybir.dt.float16
fp8 = mybir.dt.float8e4


def _split_dma_waits(nc):
    """Hoist semaphore waits onto standalone EventSemaphore instructions on
    the issuing engine's queue, for any instruction carrying more than one.

    This toolchain's walrus caps sync waits at 1 per instruction (2 for
    EventSemaphore). A sequencer executes an attached wait and a preceding
    standalone wait identically, so hoisting preserves semantics (raw-bass
    wait_ge emits exactly this instruction).
    """
    ctr = 0
    for fn in nc.m.functions:
        for bb in fn.blocks:
            new_insts = []
            for inst in bb.instructions:
                si = inst.sync_info
                if (
                    not isinstance(inst, mybir.InstEventSemaphore)
                    and si is not None
                    and len(si.on_wait) > 1
                ):
                    for w in si.on_wait[:-1]:
                        ev = mybir.InstEventSemaphore(
                            name=f"hoistw-{ctr}",
                            engine=inst.engine,
                            ins=[],
                            outs=[],
                            sync_info=mybir.SyncInfo(on_wait=[w], on_update=[]),
                        )
                        ctr += 1
                        new_insts.append(ev)
                    inst.sync_info = mybir.SyncInfo(
                        on_wait=[si.on_wait[-1]], on_update=si.on_update
                    )
                new_insts.append(inst)
            bb.instructions = new_insts


def build(n_nodes: int, debug: bool = False, split_waits: bool = True):
    """Build the SPMD Bass program for one core (all cores identical)."""
    N = n_nodes
    C = N // NCORES  # own nodes per core
    KT = N // 128  # 128-row k tiles of the strip
    JT = C // 128  # 128-col j tiles (own-node blocks)
    DEG_W = min(C, 512)
    DEG_CH = C // DEG_W  # colsum free-dim chunks (<=512 each)
    XCH = max(1, N // 1024)  # X^T column chunks
    XW = N // XCH  # columns per chunk
    MPC = XW // 128  # m tiles per X^T chunk

    nc = bass.Bass()
    a_strip = nc.dram_tensor("a_strip", [N, C], fp8, kind="ExternalInput")
    xt = nc.dram_tensor("xt", [F, N], fp16, kind="ExternalInput")
    w = nc.dram_tensor("w", [F, F], fp16, kind="ExternalInput")
    bias_bc = nc.dram_tensor("bias_bc", [128, F], f32, kind="ExternalInput")
    out = nc.dram_tensor("out", [C, F], f32, kind="ExternalOutput")
    if debug:
        deg_dump = nc.dram_tensor("deg_dump", [128, KT], f32, kind="ExternalOutput")
        z_dump = nc.dram_tensor("z_dump", [N, F], f32, kind="ExternalOutput")
        degsb_dump = nc.dram_tensor("degsb_dump", [1, C], f32, kind="ExternalOutput")
        ccin_dump = nc.dram_tensor("ccin_dump", [C], f32, kind="ExternalOutput")
        ccout_dump = nc.dram_tensor("ccout_dump", [N], f32, kind="ExternalOutput")

    with tile.TileContext(nc) as tc:
        with (
            tc.tile_pool(name="persist", bufs=1) as persist,
            tc.tile_pool(name="work", bufs=2) as work,
            tc.tile_pool(name="zpsum", bufs=2, space="PSUM") as zpsum,
            tc.tile_pool(name="degpsum", bufs=1, space="PSUM") as degpsum,
            tc.tile_pool(name="outpsum", bufs=4, space="PSUM") as outpsum,
            tc.tile_pool(name="dram", bufs=1, space="DRAM") as dram,
        ):
            # ---- write-once persistent loads ----
            # pack PK k-tiles per DMA: each DMA has a fixed issue cost, so
            # fewer/bigger transfers keep the load phase bytes-bound
            # small leading packs so the colsum chain (PE-throughput-bound,
            # gates the collective) starts as early as possible
            pack_sizes = [1, 1, 2] + [4] * ((KT - 4) // 4)
            assert sum(pack_sizes) == KT
            a_pk = []
            k2view = []
            k0 = 0
            for g, pk in enumerate(pack_sizes):
                t = persist.tile([128, pk * C], fp8, name=f"a{g}")
                a_pk.append(t)
                nc.sync.dma_start(
                    out=t.rearrange("p (t c) -> p t c", t=pk),
                    in_=a_strip[k0 * 128 : (k0 + pk) * 128, :].rearrange(
                        "(t p) c -> p t c", p=128
                    ),
                )
                for i in range(pk):
                    k2view.append((g, i))
                k0 += pk

            def a_tile(k):
                """[128, C] view of k-th row-tile of the strip."""
                g, i = k2view[k]
                return a_pk[g][:, i * C : (i + 1) * C]

            w_sb = [persist.tile([128, F], fp16, name=f"w{i}") for i in range(2)]
            for i in range(2):
                nc.sync.dma_start(out=w_sb[i][:], in_=w[i * 128 : (i + 1) * 128, :])
            bias_sb = persist.tile([128, F], f32, name="bias")
            nc.sync.dma_start(out=bias_sb[:], in_=bias_bc[:])

            ones = persist.tile([128, 128], fp16, name="ones")
            nc.vector.memset(ones[:], 1.0)

            # ---- degrees of own nodes: colsum of the strip via PE.
            # ones (stationary) x A (moving); row 0 of psum = colsums.
            deg_sb = persist.tile([1, C], f32, name="deg_sb")
            deg_ps = [
                degpsum.tile([128, DEG_W], f32, name=f"deg_ps{h}")
                for h in range(DEG_CH)
            ]
            last_cs_mm = None
            for k in range(KT):
                for h in range(DEG_CH):
                    last_cs_mm = nc.tensor.matmul(
                        deg_ps[h][:],
                        ones[:],
                        a_tile(k)[:, h * DEG_W : (h + 1) * DEG_W],
                        start=(k == 0),
                        stop=(k == KT - 1),
                    )
            for h in range(DEG_CH):
                if h % 2 == 0:
                    nc.vector.tensor_copy(
                        deg_sb[:, h * DEG_W : (h + 1) * DEG_W], deg_ps[h][0:1, :]
                    )
                else:
                    nc.scalar.copy(
                        deg_sb[:, h * DEG_W : (h + 1) * DEG_W], deg_ps[h][0:1, :]
                    )

            # ---- Z = X @ W (fp16 in, f32 accum, fp16 out), unscaled.
            # X^T streamed in write-once chunks that die after their Z-MMs.
            z_sb = [persist.tile([128, F], fp16, name=f"z{m}") for m in range(KT)]
            for ch in range(XCH):
                xt_ch = [
                    work.tile(
                        [128, XW], fp16, name=f"xt_{ch}_{i}", tag=f"xt{i}", bufs=4
                    )
                    for i in range(2)
                ]
                for i in range(2):
                    nc.sync.dma_start(
                        out=xt_ch[i][:],
                        in_=xt[i * 128 : (i + 1) * 128, ch * XW : (ch + 1) * XW],
                    )
                for mi in range(MPC):
                    m = ch * MPC + mi
                    z_ps = zpsum.tile([128, F], f32, tag="z_ps")
                    for i in range(2):
                        mm = nc.tensor.matmul(
                            z_ps[:],
                            xt_ch[i][:, mi * 128 : (mi + 1) * 128],
                            w_sb[i][:],
                            start=(i == 0),
                            stop=(i == 1),
                        )
                        # keep Z-MMs off the PE until the colsum chain (which
                        # gates the collective -> critical path) is done
                        bass._add_dep_helper(
                            mm.ins, last_cs_mm.ins, reason="z after colsum"
                        )
                    # alternate psum->sbuf drain between DVE and ACT
                    if m % 2 == 0:
                        nc.vector.tensor_copy(z_sb[m][:], z_ps[:])
                    else:
                        nc.scalar.copy(z_sb[m][:], z_ps[:])

            if debug:
                nc.sync.dma_start(out=degsb_dump[:], in_=deg_sb[0:1, :])

            # ---- gather degrees, d = 1/sqrt(deg) ----
            cc_in = dram.tile([C], f32, name="cc_in")
            cc_out = dram.tile([N], f32, name="cc_out")
            nc.sync.dma_start(
                out=cc_in.rearrange("(a b) -> a b", a=1), in_=deg_sb[0:1, :]
            )
            nc.gpsimd.collective_compute(
                "AllGather",
                mybir.AluOpType.bypass,
                replica_groups=[list(range(NCORES))],
                ins=[cc_in[:]],
                outs=[cc_out[:]],
            )
            if debug:
                nc.sync.dma_start(out=ccin_dump[:], in_=cc_in[:])
                nc.sync.dma_start(out=ccout_dump[:], in_=cc_out[:])
            deg_full = work.tile([128, KT], f32, tag="deg_full")
            nc.sync.dma_start(
                out=deg_full[:], in_=cc_out.rearrange("(k p) -> p k", p=128)
            )
            d_full = persist.tile([128, KT], f32, name="d_full")
            nc.vector.reciprocal(d_full[:], deg_full[:])
            nc.scalar.sqrt(d_full[:], d_full[:])

            # d for own nodes, [128, JT], from the local (pre-gather) degrees
            deg_own = work.tile([128, JT], f32, tag="deg_own")
            nc.sync.dma_start(
                out=deg_own[:], in_=cc_in.rearrange("(j p) -> p j", p=128)
            )
            d_own = persist.tile([128, JT], f32, name="d_own")
            nc.vector.reciprocal(d_own[:], deg_own[:])
            nc.scalar.sqrt(d_own[:], d_own[:])

            if debug:
                nc.sync.dma_start(out=deg_dump[:], in_=deg_full[:])

            # ---- Zd = d * Z, in place (alternate DVE / ACT) ----
            for m in range(KT):
                if m % 2 == 0:
                    nc.vector.tensor_scalar_mul(
                        z_sb[m][:], z_sb[m][:], d_full[:, m : m + 1]
                    )
                else:
                    nc.scalar.activation(
                        z_sb[m][:],
                        z_sb[m][:],
                        mybir.ActivationFunctionType.Copy,
                        scale=d_full[:, m : m + 1],
                    )
                if debug:
                    zs = work.tile([128, F], f32, tag="zdump")
                    nc.vector.tensor_copy(zs[:], z_sb[m][:])
                    nc.sync.dma_start(
                        out=z_dump[m * 128 : (m + 1) * 128, :], in_=zs[:]
                    )

            # ---- big matmul: out[j, f] += A[i, j] * Zd[i, f].
            # jt-outer / k-inner: one accumulation group per bank at a time.
            for jt in range(JT):
                out_ps = outpsum.tile([128, F], f32, tag="out_ps")
                for k in range(KT):
                    nc.tensor.matmul(
                        out_ps[:],
                        a_tile(k)[:, jt * 128 : (jt + 1) * 128],
                        z_sb[k][:],
                        start=(k == 0),
                        stop=(k == KT - 1),
                    )
                # epilogue: out = d_own * psum + bias
                sc = work.tile([128, F], f32, tag="sc", bufs=3)
                nc.vector.tensor_scalar_mul(sc[:], out_ps[:], d_own[:, jt : jt + 1])
                ot = work.tile([128, F], f32, tag="ot", bufs=3)
                nc.vector.tensor_tensor(
                    ot[:], sc[:], bias_sb[:], mybir.AluOpType.add
                )
                nc.sync.dma_start(out=out[jt * 128 : (jt + 1) * 128, :], in_=ot[:])

    if split_waits:
        _split_dma_waits(nc)
    return nc


_CACHE = {}


def _get_program(n_nodes: int, debug: bool = False):
    key = (n_nodes, debug)
    if key not in _CACHE:
        _CACHE[key] = build(n_nodes, debug=debug)
    return _CACHE[key]


def _prep_inputs(A, inputs, weight, bias):
    """Host-side marshaling: shard + layout + dtype casts."""
    N = A.shape[0]
    C = N // NCORES
    A_aug = np.asarray(A, dtype=np.float32)
    idx = np.arange(N)
    A_aug = A_aug.astype(ml_dtypes.float8_e4m3)
    A_aug[idx, idx] = np.float32(1.0)  # reference adds I; A diag is 0
    xt = np.ascontiguousarray(np.asarray(inputs, dtype=np.float32).T).astype(np.float16)
    w16 = np.asarray(weight, dtype=np.float32).astype(np.float16)
    bias_bc = np.ascontiguousarray(
        np.broadcast_to(np.asarray(bias, dtype=np.float32), (128, F))
    )
    in_maps = [
        {
            "a_strip": np.ascontiguousarray(A_aug[:, c * C : (c + 1) * C]),
            "xt": xt,
            "w": w16,
            "bias_bc": bias_bc,
        }
        for c in range(NCORES)
    ]
    return in_maps


def kernel(A, inputs, weight, bias):
    N = A.shape[0]
    nc = _get_program(N)
    in_maps = _prep_inputs(A, inputs, weight, bias)
    res = run_bass_kernel_spmd(nc, in_maps, list(range(NCORES)))
    return np.concatenate([r["out"] for r in res.results], axis=0)


if __name__ == "__main__":
    # mini self-check with a host reference
    N = 1024
    rng = np.random.default_rng(0)
    A = (rng.random((N, N)) < 0.01).astype(np.float32)
    A = np.maximum(A, A.T)
    np.fill_diagonal(A, 0.0)
    X = rng.standard_normal((N, F)).astype(np.float32)
    W = (rng.random((F, F)).astype(np.float32) / 100.0) - 0.005
    b = (rng.random(F).astype(np.float32) / 100.0) - 0.005

    A_ = A + np.eye(N, dtype=np.float32)
    deg = A_.sum(axis=1)
    d = deg**-0.5
    expected = (d[:, None] * A_ * d[None, :]) @ X @ W + b

    nc = _get_program(N, debug=True)
    in_maps = _prep_inputs(A, X, W, b)
    res = run_bass_kernel_spmd(nc, in_maps, list(range(NCORES)))
    r0 = res.results[0]
    deg_got = r0["deg_dump"]  # [128, KT] col k = deg[k*128:(k+1)*128]
    deg_exp = deg.reshape(-1, 128).T
    print("deg ok:", np.allclose(deg_got, deg_exp))
    z_got = r0["z_dump"]
    z_exp = d[:, None] * (X @ W)
    zerr = np.abs(z_got - z_exp) / (np.abs(z_exp).max())
    print("zd rel err:", zerr.max(), "nan:", np.isnan(z_got).sum())

    got = np.concatenate([r["out"] for r in res.results], axis=0)
    err = np.abs(got - expected)
    scale = np.abs(expected).max()
    print("rel err:", err.max() / scale, "nan:", np.isnan(got).sum(), "/", got.size)
